# revision 13
# baseline (speedup 1.0000x reference)
"""GNN message-passing Bass kernel for TRN2 (8 cores, SPMD).

Math (reference):
  h0 = segsum_dst(w_e * feature[src_e])              # [N, 128]
  for t in 0..3:
    h  = relu(h0 @ (layer1*mask1[t]))                # [N, 128]
    p_t = h @ (layer2*mask2[t])                      # [N, 16]
  out_t = segsum_dst(w_e * p_t[src_e])               # [N, 16]  (A @ p_t)

Key transformation: out_t = A @ (h_t @ W2_t) so the second aggregation runs on
16-wide vectors (64 for all t stacked), not 128-wide.

Implementation: two launches.
  Launch A: edge-gather from bf16 feature table (HBM), scatter via per-tile
    matmul  h0T[f, win] += M_tile.T @ S'_tile  (feature-major accumulation in
    PSUM), then the dense GEMMs (fp32) -> pT staged [128, NP] (rows 32t+o),
    fused per 512-column group so the GEMM overlaps later groups' gathers.
  Host: assemble p-table [50176, 128] bf16 (64 values + 64 zero pad per row).
  Launch B: same aggregation structure against the p-table -> out2T [64, NP].

Edges are partitioned by dst across cores (6250 nodes each); each tile of 128
edges belongs to one 64-node dst window and one src bucket (src < 32768 or
not, because gather indices are int16). The SWDGE gather (extended-inst
DMAGatherAnt on the Pool engine) is the kernel bottleneck at ~8.5ns per
128-token chunk-slot of descriptor generation; invalid lanes still emit dummy
descriptors, so the floor is the tile count and all pad slots are plain valid
tokens (index 0, weight 0). Tile counts per (group, window, bucket) are padded
to the max across cores so one SPMD program serves all 8.
"""

import sys

sys.path.insert(0, "/opt/trn_rl_repo")

import numpy as np
import ml_dtypes

import concourse.bass as bass
import concourse.bacc as bacc
import concourse.mybir as mybir
import concourse.tile as tile

F32 = mybir.dt.float32
BF16 = mybir.dt.bfloat16
I16 = mybir.dt.int16

TILE = 128          # edges per tile
W = 64              # dst nodes per window (matmul moving width)
GROUP_W = 8         # windows per psum group (8*64 = 512 fp32 cols = 1 bank)
OP_TILES = 8        # max tiles per dma_gather op (1024 tokens; >=1536 tokens
                    # per op overflows the SWDGE descriptor ring on HW)
SPLIT = 32768       # int16 index split
GBUFS = 8           # gather-buffer rotation depth


# ---------------------------------------------------------------------------
# Host-side planning
# ---------------------------------------------------------------------------

class Plan:
    """Uniform (cross-core) tile plan for one aggregation.

    real_max: [ngroups][2 buckets][GROUP_W windows] -> max (over cores) real
    token count in that (window, bucket) segment.

    Tiles are packed per (group, bucket) run with uniform per-window slot
    offsets (cross-core max counts), so tiles may span window boundaries;
    a tile issues one matmul per window it overlaps. This removes the
    per-window ceil-to-128 padding that costs SWDGE descriptor-generation
    time (the kernel bottleneck).
    """

    def __init__(self, n_nodes, real_max):
        self.n_nodes = n_nodes
        self.nwin = -(-n_nodes // W)
        self.n_nodes_pad = self.nwin * W
        self.ngroups = -(-self.nwin // GROUP_W)
        self.real_max = real_max
        self.tile_bucket = []
        self.groups = []     # per group: dict(ops=[...], c0, c1)
        self.run_info = {}   # (g, b) -> (slot_base, offs[GROUP_W], cnts[GROUP_W])
        c = 0
        nblk = 0
        for g in range(self.ngroups):
            c0g = c
            ops = []
            mms_group = []   # (tile c, wl, blk)
            for b in range(2):
                cnts = [int(real_max[g][b][wi]) for wi in range(GROUP_W)]
                total = sum(cnts)
                if total == 0:
                    continue
                ntiles = -(-total // TILE)
                run_c0 = c
                offs = [0] * GROUP_W
                for wi in range(1, GROUP_W):
                    offs[wi] = offs[wi - 1] + cnts[wi - 1]
                self.run_info[(g, b)] = (run_c0 * TILE, offs, cnts)
                tile_mms = []
                for t in range(ntiles):
                    s0, s1 = t * TILE, (t + 1) * TILE
                    mms = []
                    for wi in range(GROUP_W):
                        if cnts[wi] == 0:
                            continue
                        if offs[wi] < s1 and offs[wi] + cnts[wi] > s0:
                            mms.append((run_c0 + t, wi, nblk))
                            nblk += 1
                    tile_mms.append(mms)
                    self.tile_bucket.append(b)
                c += ntiles
                # chunk the run into gather ops
                i = 0
                while i < ntiles:
                    n = min(OP_TILES, ntiles - i)
                    op_mms = [m for tm in tile_mms[i:i + n] for m in tm]
                    blk0 = op_mms[0][2]
                    ops.append({
                        "c0": run_c0 + i, "n": n,
                        "blk0": blk0, "nblk": len(op_mms),
                        "mms": [(tc - (run_c0 + i), wl, bk - blk0)
                                for (tc, wl, bk) in op_mms],
                    })
                    mms_group.extend(op_mms)
                    i += n
            assert mms_group, f"group {g} has no matmuls"
            self.groups.append({"ops": ops, "c0": c0g, "c1": c,
                                "n_mms": len(mms_group)})
        self.nt = c
        self.nblk = nblk


def count_core(srct, dstloc, n_nodes):
    """Per-core real token counts [ngroups][2][GROUP_W]."""
    nwin = -(-n_nodes // W)
    ngroups = -(-nwin // GROUP_W)
    win = dstloc // W
    bucket = (srct >= SPLIT).astype(np.int64)
    cnt = np.zeros((ngroups, 2, GROUP_W), np.int64)
    key = (win * 2 + bucket).astype(np.int64)
    bc = np.bincount(key, minlength=nwin * 2)
    for gw in range(nwin):
        g, wi = divmod(gw, GROUP_W)
        for b in range(2):
            cnt[g][b][wi] = bc[gw * 2 + b]
    return cnt


def merge_counts(all_counts):
    return np.maximum.reduce(all_counts)


def build_core_data(plan: Plan, srct, dstloc, wgt):
    """Per-core idx + scatter-weight arrays matching the uniform plan.

    Every slot is a valid token (pads gather row 0 with weight 0): invalid
    lanes would still cost dummy descriptors, and num_idxs must be uniform
    across cores to keep the NX descriptor-ring accounting in sync.
    Returns idx_np [128, NT*8] int16, sw_np [128, NBLK*W] bf16.
    """
    nt, nblk = plan.nt, plan.nblk
    tok_flat = np.zeros(nt * TILE, np.int64)
    sw = np.zeros((TILE, nblk * W), np.float32)

    win = dstloc // W
    bucket = (srct >= SPLIT).astype(np.int64)
    order = np.lexsort((srct, win * 2 + bucket))
    s_srct = srct[order]
    s_dstloc = dstloc[order]
    s_w = wgt[order].astype(np.float32)
    s_key = (win * 2 + bucket)[order]

    bounds = np.flatnonzero(np.r_[True, s_key[1:] != s_key[:-1], True])
    seg = {}
    for a, b in zip(bounds[:-1], bounds[1:]):
        seg[int(s_key[a])] = (int(a), int(b))

    # (tile c, window wl) -> sw block id
    blk_of = {}
    for grp in plan.groups:
        for op in grp["ops"]:
            for (i, wl, bl) in op["mms"]:
                blk_of[(op["c0"] + i, wl)] = op["blk0"] + bl

    for (g, b), (slot_base, offs, cnts) in plan.run_info.items():
        for wi in range(GROUP_W):
            if cnts[wi] == 0:
                continue
            gw = g * GROUP_W + wi
            a, e = seg.get(gw * 2 + b, (0, 0))
            n = e - a
            assert n <= cnts[wi], f"plan too small for seg {(gw, b)}"
            if n == 0:
                continue
            s0 = slot_base + offs[wi]
            tok_flat[s0:s0 + n] = s_srct[a:e] - b * SPLIT
            slots = np.arange(s0, s0 + n)
            p = slots % TILE
            tc = slots // TILE
            blks = np.array([blk_of[(c, wi)] for c in np.unique(tc)])
            blk_per_slot = np.array([blk_of[(int(c), wi)] for c in tc])
            cols = s_dstloc[a:e] - gw * W
            sw[p, blk_per_slot * W + cols] = s_w[a:e]

    ni = tok_flat.shape[0]
    idx_np = np.tile(tok_flat.reshape(ni // 16, 16).T, (8, 1)).astype(np.int16)
    sw_np = np.ascontiguousarray(sw).astype(ml_dtypes.bfloat16)
    return idx_np, sw_np


# ---------------------------------------------------------------------------
# Device-side emit
# ---------------------------------------------------------------------------

def emit_aggregation(tc, nc, plan: Plan, table_lo, table_hi, idx_dram, sw_dram,
                     out_sbuf, out_rows, elem=128, per_group=None):
    """Gather + matmul-scatter. out_sbuf [>=out_rows, ngroups*512] fp32."""
    MAXBLK = OP_TILES + GROUP_W - 1
    with (
        tc.tile_pool(name="agg_idx", bufs=1) as ipool,
        tc.tile_pool(name="agg_g", bufs=GBUFS) as gpool,
        tc.tile_pool(name="agg_s", bufs=12) as spool,
        tc.tile_pool(name="agg_ps", bufs=3, space="PSUM") as pspool,
    ):
        ni = plan.nt * TILE
        idx_t = ipool.tile([128, ni // 16], I16)
        quarter = (ni // 16) // 4
        nc.sync.dma_start(out=idx_t[:, :quarter], in_=idx_dram[:, :quarter])
        nc.sync.dma_start(out=idx_t[:, quarter:], in_=idx_dram[:, quarter:])
        for g, grp in enumerate(plan.groups):
            ps = pspool.tile([128, GROUP_W * W], F32)
            mm_seen = 0
            for op in grp["ops"]:
                c0, n, blk0, nblk = op["c0"], op["n"], op["blk0"], op["nblk"]
                assert nblk <= MAXBLK
                b = plan.tile_bucket[c0]
                gd = gpool.tile([128, OP_TILES, elem], BF16)
                swt = spool.tile([128, MAXBLK * W], BF16)
                nc.sync.dma_start(
                    out=swt[:, : nblk * W],
                    in_=sw_dram[:, blk0 * W:(blk0 + nblk) * W],
                )
                nc.gpsimd.dma_gather(
                    out_ap=gd[:, :n, :],
                    in_ap=(table_hi if b else table_lo),
                    idxs_ap=idx_t[:, c0 * 8:(c0 + n) * 8],
                    num_idxs=n * TILE,
                    num_idxs_reg=n * TILE,
                    elem_size=elem,
                )
                for (i, wl, bl) in op["mms"]:
                    mm_seen += 1
                    nc.tensor.matmul(
                        out=ps[:, wl * W:(wl + 1) * W],
                        lhsT=gd[:, i, :],
                        rhs=swt[:, bl * W:(bl + 1) * W],
                        start=(mm_seen == 1),
                        stop=(mm_seen == grp["n_mms"]),
                    )
            nc.vector.tensor_copy(
                out=out_sbuf[:out_rows, g * GROUP_W * W:(g + 1) * GROUP_W * W],
                in_=ps[:out_rows, :],
            )
            if per_group is not None:
                per_group(g)


def build_launch_a(plan: Plan, n_table_rows):
    """Launch A: aggregation-1 + GEMMs -> pt [128, NP] (rows 32t+o used)."""
    np_pad = plan.ngroups * GROUP_W * W
    nc = bacc.Bacc("TRN2", target_bir_lowering=False, debug=False, num_devices=8)
    ftab = nc.dram_tensor("ftab", [n_table_rows, 128], BF16, kind="ExternalInput")
    idx_d = nc.dram_tensor("idx", [128, plan.nt * 8], I16, kind="ExternalInput")
    sw_d = nc.dram_tensor("sw", [128, plan.nblk * W], BF16, kind="ExternalInput")
    l1_d = nc.dram_tensor("l1", [4, 128, 128], F32, kind="ExternalInput")  # premasked
    l2_d = nc.dram_tensor("l2", [4, 128, 32], F32, kind="ExternalInput")   # premasked+padded
    pt_d = nc.dram_tensor("pt", [128, np_pad], F32, kind="ExternalOutput")

    hb = SPLIT if n_table_rows > SPLIT else 0
    with tile.TileContext(nc) as tc:
        with (
            tc.tile_pool(name="h0", bufs=1) as h0pool,
            tc.tile_pool(name="wts", bufs=1) as wpool,
            tc.tile_pool(name="hs", bufs=3) as hspool,
            tc.tile_pool(name="ptst", bufs=1) as ptpool,
            tc.tile_pool(name="ps1", bufs=2, space="PSUM") as ps1pool,
            tc.tile_pool(name="ps2", bufs=2, space="PSUM") as ps2pool,
        ):
            h0T = h0pool.tile([128, np_pad], F32)
            w1 = wpool.tile([128, 4, 128], F32)
            nc.sync.dma_start(out=w1[:], in_=l1_d.rearrange("t k h -> k t h"))
            w2 = wpool.tile([128, 4, 32], F32)
            nc.sync.dma_start(out=w2[:], in_=l2_d.rearrange("t k h -> k t h"))
            ptst = ptpool.tile([128, np_pad], F32)

            def per_group(ch):
                sl = slice(ch * 512, (ch + 1) * 512)
                ps2 = ps2pool.tile([128, 512], F32)
                for t in range(4):
                    ps1 = ps1pool.tile([128, 512], F32)
                    nc.tensor.matmul(out=ps1[:], lhsT=w1[:, t, :], rhs=h0T[:, sl],
                                     start=True, stop=True)
                    hs = hspool.tile([128, 512], F32)
                    nc.scalar.activation(
                        out=hs[:], in_=ps1[:],
                        func=mybir.ActivationFunctionType.Relu,
                    )
                    nc.tensor.matmul(out=ps2[32 * t:32 * t + 32, :],
                                     lhsT=w2[:, t, :], rhs=hs[:],
                                     start=True, stop=True,
                                     tile_position=(0, 32 * t))
                nc.vector.tensor_copy(out=ptst[:, sl], in_=ps2[:])
                nc.sync.dma_start(out=pt_d[:, sl], in_=ptst[:, sl])

            emit_aggregation(tc, nc, plan, ftab[:min(SPLIT, n_table_rows), :],
                             ftab[hb:, :], idx_d, sw_d, h0T, 128,
                             per_group=per_group)
    nc.compile()
    return nc


def build_launch_b(plan: Plan, n_table_rows):
    """Launch B: aggregation-2 against p-table -> o2 [64, NP]."""
    np_pad = plan.ngroups * GROUP_W * W
    nc = bacc.Bacc("TRN2", target_bir_lowering=False, debug=False, num_devices=8)
    ptab = nc.dram_tensor("ptab", [n_table_rows, 128], BF16, kind="ExternalInput")
    idx_d = nc.dram_tensor("idx", [128, plan.nt * 8], I16, kind="ExternalInput")
    sw_d = nc.dram_tensor("sw", [128, plan.nblk * W], BF16, kind="ExternalInput")
    o2_d = nc.dram_tensor("o2", [64, np_pad], F32, kind="ExternalOutput")

    hb = SPLIT if n_table_rows > SPLIT else 0
    with tile.TileContext(nc) as tc:
        with tc.tile_pool(name="o2", bufs=1) as opool:
            o2 = opool.tile([64, np_pad], F32)
            emit_aggregation(tc, nc, plan, ptab[:min(SPLIT, n_table_rows), :],
                             ptab[hb:, :], idx_d, sw_d, o2, 64)
            nc.sync.dma_start(out=o2_d[:], in_=o2[:])
    nc.compile()
    return nc


# ---------------------------------------------------------------------------
# Runners
# ---------------------------------------------------------------------------

def sim_runner(nc, in_maps):
    from concourse.bass_interp import CoreSim
    outs = []
    for m in in_maps:
        sim = CoreSim(nc, trace=False, require_finite=False, require_nnan=False)
        for name, val in m.items():
            sim.tensor(name)[:] = val
        sim.simulate(check_with_hw=False)
        out = {}
        for alloc in nc.m.functions[0].allocations:
            if isinstance(alloc, mybir.MemoryLocationSet) and alloc.kind == "ExternalOutput":
                name = alloc.memorylocations[0].name
                out[name] = np.array(sim.tensor(name))
        outs.append(out)
    return outs


def _install_ntff_hook():
    """The agent image's antenv lacks axon_hooks; synthesize it so
    run_bass_kernel_spmd(trace=True) can NTFF-profile via the axon .so."""
    import types
    if "antenv.axon_hooks" in sys.modules:
        return True
    try:
        from trn_agent_boot.trn_boot import _ntff_profile_via_ctypes
        hook = _ntff_profile_via_ctypes("/opt/axon/libaxon_pjrt.so")
    except Exception:
        return False
    mod = types.ModuleType("antenv.axon_hooks")
    mod._hook = hook
    mod.set_axon_ntff_profile_hook = lambda h: setattr(mod, "_hook", h)
    mod.get_axon_ntff_profile_hook = lambda: mod._hook
    sys.modules["antenv.axon_hooks"] = mod
    try:
        import antenv
        antenv.axon_hooks = mod
    except Exception:
        pass
    return True


def hw_runner_factory(trace=False, label=""):
    from concourse.bass_utils import run_bass_kernel_spmd
    if trace:
        trace = _install_ntff_hook()
    times = {}

    def hw_runner(nc, in_maps):
        res = run_bass_kernel_spmd(nc, in_maps, core_ids=list(range(len(in_maps))),
                                   trace=trace)
        times[label or "t"] = times.get(label or "t", 0) + (res.exec_time_ns or 0)
        hw_runner.last = res
        return res.results

    hw_runner.times = times
    return hw_runner


# ---------------------------------------------------------------------------
# Full host orchestration
# ---------------------------------------------------------------------------

def run(feature, edge_weight, layer1, layer2, src, dst, mask1, mask2,
        n_cores=8, runner=None, trace=False):
    """runner(nc, in_maps) -> list of out dicts; defaults to HW spmd."""
    N = feature.shape[0]
    E = src.shape[0]
    T = mask1.shape[0]
    npc = -(-N // n_cores)          # nodes per core
    nrows = ((N + 127) // 128) * 128
    src = np.asarray(src).astype(np.int64)
    dst = np.asarray(dst).astype(np.int64)
    w = np.asarray(edge_weight).astype(np.float32)

    core_of = dst // npc
    per_core = []
    for k in range(n_cores):
        m = core_of == k
        per_core.append((src[m], dst[m] - k * npc, w[m]))

    real_max = merge_counts([count_core(s, d, npc) for (s, d, _) in per_core])
    plan = Plan(npc, real_max)

    idx_all, sw_all = [], []
    for k in range(n_cores):
        s, d, ww = per_core[k]
        idx_np, sw_np = build_core_data(plan, s, d, ww)
        idx_all.append(idx_np)
        sw_all.append(sw_np)

    # feature table bf16 [nrows, 128]
    ftab = np.zeros((nrows, 128), ml_dtypes.bfloat16)
    ftab[:N] = feature.astype(ml_dtypes.bfloat16)

    # premasked weights
    l1m = (np.asarray(layer1)[None] * np.asarray(mask1)).astype(np.float32)
    l2m = np.zeros((T, 128, 32), np.float32)
    l2m[:, :, :16] = np.asarray(layer2)[None] * np.asarray(mask2)

    nc_a = build_launch_a(plan, nrows)
    in_maps_a = [
        {"ftab": ftab, "idx": idx_all[k], "sw": sw_all[k], "l1": l1m, "l2": l2m}
        for k in range(n_cores)
    ]
    res_a = runner(nc_a, in_maps_a)

    # assemble p-table: rows n -> 64 p values (r = 32t + o from pt rows)
    np_pad = plan.ngroups * GROUP_W * W
    ptab = np.zeros((nrows, 128), ml_dtypes.bfloat16)
    for k in range(n_cores):
        pt = res_a[k]["pt"]  # [128, np_pad]
        rows = np.concatenate([pt[32 * t:32 * t + 16] for t in range(T)])  # [64, NP]
        n0, n1 = k * npc, min((k + 1) * npc, N)
        ptab[n0:n1, :64] = rows[:, : n1 - n0].T.astype(ml_dtypes.bfloat16)

    nc_b = build_launch_b(plan, nrows)
    in_maps_b = [
        {"ptab": ptab, "idx": idx_all[k], "sw": sw_all[k]}
        for k in range(n_cores)
    ]
    res_b = runner(nc_b, in_maps_b)

    out = np.zeros((T, N, 16), np.float32)
    for k in range(n_cores):
        o2 = res_b[k]["o2"]  # [64, np_pad]
        n0, n1 = k * npc, min((k + 1) * npc, N)
        blk = o2[:, : n1 - n0].reshape(T, 16, n1 - n0)
        out[:, n0:n1, :] = blk.transpose(0, 2, 1)
    return out


# ---------------------------------------------------------------------------
# Harness entry point
# ---------------------------------------------------------------------------

def kernel(feature, edge_weight, layer1, layer2, src, dst, mask1, mask2):
    """Full (unsharded) inputs -> full [T, N, 16] float32 output.

    Shards edges by dst range across 8 NeuronCores, runs two Bass launches
    (aggregation-1 + GEMMs, then aggregation-2), gathers on host.
    """
    import os
    trace = bool(os.environ.get("KERNEL_TRACE"))
    runner = hw_runner_factory(trace=trace)
    out = run(
        np.asarray(feature, np.float32),
        np.asarray(edge_weight, np.float32),
        np.asarray(layer1, np.float32),
        np.asarray(layer2, np.float32),
        np.asarray(src),
        np.asarray(dst),
        np.asarray(mask1),
        np.asarray(mask2),
        n_cores=8,
        runner=runner,
    )
    kernel.exec_time_ns = sum(runner.times.values()) if trace else None
    return out


# revision 14
# speedup vs baseline: 1.0211x; 1.0211x over previous
"""GNN message-passing Bass kernel for TRN2 (8 cores, SPMD).

Math (reference):
  h0 = segsum_dst(w_e * feature[src_e])              # [N, 128]
  for t in 0..3:
    h  = relu(h0 @ (layer1*mask1[t]))                # [N, 128]
    p_t = h @ (layer2*mask2[t])                      # [N, 16]
  out_t = segsum_dst(w_e * p_t[src_e])               # [N, 16]  (A @ p_t)

Key transformation: out_t = A @ (h_t @ W2_t) so the second aggregation runs on
16-wide vectors (64 for all t stacked), not 128-wide.

Implementation: two launches.
  Launch A: edge-gather from bf16 feature table (HBM), scatter via per-tile
    matmul  h0T[f, win] += M_tile.T @ S'_tile  (feature-major accumulation in
    PSUM), then the dense GEMMs (fp32) -> pT staged [128, NP] (rows 32t+o),
    fused per 512-column group so the GEMM overlaps later groups' gathers.
  Host: assemble p-table [50176, 128] bf16 (64 values + 64 zero pad per row).
  Launch B: same aggregation structure against the p-table -> out2T [64, NP].

Edges are partitioned by dst across cores (6250 nodes each); each tile of 128
edges belongs to one 64-node dst window and one src bucket (src < 32768 or
not, because gather indices are int16). The SWDGE gather (extended-inst
DMAGatherAnt on the Pool engine) is the kernel bottleneck at ~8.5ns per
128-token chunk-slot of descriptor generation; invalid lanes still emit dummy
descriptors, so the floor is the tile count and all pad slots are plain valid
tokens (index 0, weight 0). Tile counts per (group, window, bucket) are padded
to the max across cores so one SPMD program serves all 8.
"""

import sys

sys.path.insert(0, "/opt/trn_rl_repo")

import numpy as np
import ml_dtypes

import concourse.bass as bass
import concourse.bacc as bacc
import concourse.mybir as mybir
import concourse.tile as tile

F32 = mybir.dt.float32
BF16 = mybir.dt.bfloat16
I16 = mybir.dt.int16

TILE = 128          # edges per tile
W = 256             # dst nodes per window (matmul moving width)
GROUP_W = 2         # windows per psum group (2*256 = 512 fp32 cols = 1 bank)
OP_TILES = 8        # max tiles per dma_gather op (1024 tokens; >=1536 tokens
                    # per op overflows the SWDGE descriptor ring on HW)
SPLIT = 32768       # int16 index split
GBUFS = 8           # gather-buffer rotation depth


# ---------------------------------------------------------------------------
# Host-side planning
# ---------------------------------------------------------------------------

class Plan:
    """Uniform (cross-core) tile plan for one aggregation.

    real_max: [ngroups][2 buckets][GROUP_W windows] -> max (over cores) real
    token count in that (window, bucket) segment.

    Tiles are packed per (group, bucket) run with uniform per-window slot
    offsets (cross-core max counts), so tiles may span window boundaries;
    a tile issues one matmul per window it overlaps. This removes the
    per-window ceil-to-128 padding that costs SWDGE descriptor-generation
    time (the kernel bottleneck).
    """

    def __init__(self, n_nodes, real_max):
        self.n_nodes = n_nodes
        self.nwin = -(-n_nodes // W)
        self.n_nodes_pad = self.nwin * W
        self.ngroups = -(-self.nwin // GROUP_W)
        self.real_max = real_max
        self.tile_bucket = []
        self.groups = []     # per group: dict(ops=[...], c0, c1)
        self.run_info = {}   # (g, b) -> (slot_base, offs[GROUP_W], cnts[GROUP_W])
        c = 0
        nblk = 0
        for g in range(self.ngroups):
            c0g = c
            ops = []
            mms_group = []   # (tile c, wl, blk)
            for b in range(2):
                cnts = [int(real_max[g][b][wi]) for wi in range(GROUP_W)]
                total = sum(cnts)
                if total == 0:
                    continue
                ntiles = -(-total // TILE)
                run_c0 = c
                offs = [0] * GROUP_W
                for wi in range(1, GROUP_W):
                    offs[wi] = offs[wi - 1] + cnts[wi - 1]
                self.run_info[(g, b)] = (run_c0 * TILE, offs, cnts)
                tile_mms = []
                for t in range(ntiles):
                    s0, s1 = t * TILE, (t + 1) * TILE
                    mms = []
                    for wi in range(GROUP_W):
                        if cnts[wi] == 0:
                            continue
                        if offs[wi] < s1 and offs[wi] + cnts[wi] > s0:
                            mms.append((run_c0 + t, wi, nblk))
                            nblk += 1
                    tile_mms.append(mms)
                    self.tile_bucket.append(b)
                c += ntiles
                # chunk the run into gather ops
                i = 0
                while i < ntiles:
                    n = min(OP_TILES, ntiles - i)
                    op_mms = [m for tm in tile_mms[i:i + n] for m in tm]
                    blk0 = op_mms[0][2]
                    ops.append({
                        "c0": run_c0 + i, "n": n,
                        "blk0": blk0, "nblk": len(op_mms),
                        "mms": [(tc - (run_c0 + i), wl, bk - blk0)
                                for (tc, wl, bk) in op_mms],
                    })
                    mms_group.extend(op_mms)
                    i += n
            assert mms_group, f"group {g} has no matmuls"
            self.groups.append({"ops": ops, "c0": c0g, "c1": c,
                                "n_mms": len(mms_group)})
        self.nt = c
        self.nblk = nblk


def count_core(srct, dstloc, n_nodes):
    """Per-core real token counts [ngroups][2][GROUP_W]."""
    nwin = -(-n_nodes // W)
    ngroups = -(-nwin // GROUP_W)
    win = dstloc // W
    bucket = (srct >= SPLIT).astype(np.int64)
    cnt = np.zeros((ngroups, 2, GROUP_W), np.int64)
    key = (win * 2 + bucket).astype(np.int64)
    bc = np.bincount(key, minlength=nwin * 2)
    for gw in range(nwin):
        g, wi = divmod(gw, GROUP_W)
        for b in range(2):
            cnt[g][b][wi] = bc[gw * 2 + b]
    return cnt


def merge_counts(all_counts):
    return np.maximum.reduce(all_counts)


def build_core_data(plan: Plan, srct, dstloc, wgt):
    """Per-core idx + scatter-weight arrays matching the uniform plan.

    Every slot is a valid token (pads gather row 0 with weight 0): invalid
    lanes would still cost dummy descriptors, and num_idxs must be uniform
    across cores to keep the NX descriptor-ring accounting in sync.
    Returns idx_np [128, NT*8] int16, sw_np [128, NBLK*W] bf16.
    """
    nt, nblk = plan.nt, plan.nblk
    tok_flat = np.zeros(nt * TILE, np.int64)
    sw = np.zeros((TILE, nblk * W), np.float32)

    win = dstloc // W
    bucket = (srct >= SPLIT).astype(np.int64)
    order = np.lexsort((srct, win * 2 + bucket))
    s_srct = srct[order]
    s_dstloc = dstloc[order]
    s_w = wgt[order].astype(np.float32)
    s_key = (win * 2 + bucket)[order]

    bounds = np.flatnonzero(np.r_[True, s_key[1:] != s_key[:-1], True])
    seg = {}
    for a, b in zip(bounds[:-1], bounds[1:]):
        seg[int(s_key[a])] = (int(a), int(b))

    # (tile c, window wl) -> sw block id
    blk_of = {}
    for grp in plan.groups:
        for op in grp["ops"]:
            for (i, wl, bl) in op["mms"]:
                blk_of[(op["c0"] + i, wl)] = op["blk0"] + bl

    for (g, b), (slot_base, offs, cnts) in plan.run_info.items():
        for wi in range(GROUP_W):
            if cnts[wi] == 0:
                continue
            gw = g * GROUP_W + wi
            a, e = seg.get(gw * 2 + b, (0, 0))
            n = e - a
            assert n <= cnts[wi], f"plan too small for seg {(gw, b)}"
            if n == 0:
                continue
            s0 = slot_base + offs[wi]
            tok_flat[s0:s0 + n] = s_srct[a:e] - b * SPLIT
            slots = np.arange(s0, s0 + n)
            p = slots % TILE
            tc = slots // TILE
            blks = np.array([blk_of[(c, wi)] for c in np.unique(tc)])
            blk_per_slot = np.array([blk_of[(int(c), wi)] for c in tc])
            cols = s_dstloc[a:e] - gw * W
            sw[p, blk_per_slot * W + cols] = s_w[a:e]

    ni = tok_flat.shape[0]
    idx_np = np.tile(tok_flat.reshape(ni // 16, 16).T, (8, 1)).astype(np.int16)
    sw_np = np.ascontiguousarray(sw).astype(ml_dtypes.bfloat16)
    return idx_np, sw_np


# ---------------------------------------------------------------------------
# Device-side emit
# ---------------------------------------------------------------------------

def emit_aggregation(tc, nc, plan: Plan, table_lo, table_hi, idx_dram, sw_dram,
                     out_sbuf, out_rows, elem=128, per_group=None):
    """Gather + matmul-scatter. out_sbuf [>=out_rows, ngroups*512] fp32."""
    MAXBLK = OP_TILES + GROUP_W - 1
    with (
        tc.tile_pool(name="agg_idx", bufs=1) as ipool,
        tc.tile_pool(name="agg_g", bufs=GBUFS) as gpool,
        tc.tile_pool(name="agg_s", bufs=12) as spool,
        tc.tile_pool(name="agg_ps", bufs=3, space="PSUM") as pspool,
    ):
        ni = plan.nt * TILE
        idx_t = ipool.tile([128, ni // 16], I16)
        quarter = (ni // 16) // 4
        nc.sync.dma_start(out=idx_t[:, :quarter], in_=idx_dram[:, :quarter])
        nc.sync.dma_start(out=idx_t[:, quarter:], in_=idx_dram[:, quarter:])
        for g, grp in enumerate(plan.groups):
            ps = pspool.tile([128, GROUP_W * W], F32)
            mm_seen = 0
            for op in grp["ops"]:
                c0, n, blk0, nblk = op["c0"], op["n"], op["blk0"], op["nblk"]
                assert nblk <= MAXBLK
                b = plan.tile_bucket[c0]
                gd = gpool.tile([128, OP_TILES, elem], BF16)
                swt = spool.tile([128, MAXBLK * W], BF16)
                nc.sync.dma_start(
                    out=swt[:, : nblk * W],
                    in_=sw_dram[:, blk0 * W:(blk0 + nblk) * W],
                )
                nc.gpsimd.dma_gather(
                    out_ap=gd[:, :n, :],
                    in_ap=(table_hi if b else table_lo),
                    idxs_ap=idx_t[:, c0 * 8:(c0 + n) * 8],
                    num_idxs=n * TILE,
                    num_idxs_reg=n * TILE,
                    elem_size=elem,
                )
                for (i, wl, bl) in op["mms"]:
                    mm_seen += 1
                    nc.tensor.matmul(
                        out=ps[:, wl * W:(wl + 1) * W],
                        lhsT=gd[:, i, :],
                        rhs=swt[:, bl * W:(bl + 1) * W],
                        start=(mm_seen == 1),
                        stop=(mm_seen == grp["n_mms"]),
                    )
            nc.vector.tensor_copy(
                out=out_sbuf[:out_rows, g * GROUP_W * W:(g + 1) * GROUP_W * W],
                in_=ps[:out_rows, :],
            )
            if per_group is not None:
                per_group(g)


def build_launch_a(plan: Plan, n_table_rows):
    """Launch A: aggregation-1 + GEMMs -> pt [128, NP] (rows 32t+o used)."""
    np_pad = plan.ngroups * GROUP_W * W
    nc = bacc.Bacc("TRN2", target_bir_lowering=False, debug=False, num_devices=8)
    ftab = nc.dram_tensor("ftab", [n_table_rows, 128], BF16, kind="ExternalInput")
    idx_d = nc.dram_tensor("idx", [128, plan.nt * 8], I16, kind="ExternalInput")
    sw_d = nc.dram_tensor("sw", [128, plan.nblk * W], BF16, kind="ExternalInput")
    l1_d = nc.dram_tensor("l1", [4, 128, 128], F32, kind="ExternalInput")  # premasked
    l2_d = nc.dram_tensor("l2", [4, 128, 32], F32, kind="ExternalInput")   # premasked+padded
    pt_d = nc.dram_tensor("pt", [128, np_pad], F32, kind="ExternalOutput")

    hb = SPLIT if n_table_rows > SPLIT else 0
    with tile.TileContext(nc) as tc:
        with (
            tc.tile_pool(name="h0", bufs=1) as h0pool,
            tc.tile_pool(name="wts", bufs=1) as wpool,
            tc.tile_pool(name="hs", bufs=3) as hspool,
            tc.tile_pool(name="ptst", bufs=1) as ptpool,
            tc.tile_pool(name="ps1", bufs=2, space="PSUM") as ps1pool,
            tc.tile_pool(name="ps2", bufs=2, space="PSUM") as ps2pool,
        ):
            h0T = h0pool.tile([128, np_pad], F32)
            w1 = wpool.tile([128, 4, 128], F32)
            nc.sync.dma_start(out=w1[:], in_=l1_d.rearrange("t k h -> k t h"))
            w2 = wpool.tile([128, 4, 32], F32)
            nc.sync.dma_start(out=w2[:], in_=l2_d.rearrange("t k h -> k t h"))
            ptst = ptpool.tile([128, np_pad], F32)

            def per_group(ch):
                sl = slice(ch * 512, (ch + 1) * 512)
                ps2 = ps2pool.tile([128, 512], F32)
                for t in range(4):
                    ps1 = ps1pool.tile([128, 512], F32)
                    nc.tensor.matmul(out=ps1[:], lhsT=w1[:, t, :], rhs=h0T[:, sl],
                                     start=True, stop=True)
                    hs = hspool.tile([128, 512], F32)
                    nc.scalar.activation(
                        out=hs[:], in_=ps1[:],
                        func=mybir.ActivationFunctionType.Relu,
                    )
                    nc.tensor.matmul(out=ps2[32 * t:32 * t + 32, :],
                                     lhsT=w2[:, t, :], rhs=hs[:],
                                     start=True, stop=True,
                                     tile_position=(0, 32 * t))
                nc.vector.tensor_copy(out=ptst[:, sl], in_=ps2[:])
                nc.sync.dma_start(out=pt_d[:, sl], in_=ptst[:, sl])

            emit_aggregation(tc, nc, plan, ftab[:min(SPLIT, n_table_rows), :],
                             ftab[hb:, :], idx_d, sw_d, h0T, 128,
                             per_group=per_group)
    nc.compile()
    return nc


def build_launch_b(plan: Plan, n_table_rows):
    """Launch B: aggregation-2 against p-table -> o2 [64, NP]."""
    np_pad = plan.ngroups * GROUP_W * W
    nc = bacc.Bacc("TRN2", target_bir_lowering=False, debug=False, num_devices=8)
    ptab = nc.dram_tensor("ptab", [n_table_rows, 128], BF16, kind="ExternalInput")
    idx_d = nc.dram_tensor("idx", [128, plan.nt * 8], I16, kind="ExternalInput")
    sw_d = nc.dram_tensor("sw", [128, plan.nblk * W], BF16, kind="ExternalInput")
    o2_d = nc.dram_tensor("o2", [64, np_pad], F32, kind="ExternalOutput")

    hb = SPLIT if n_table_rows > SPLIT else 0
    with tile.TileContext(nc) as tc:
        with tc.tile_pool(name="o2", bufs=1) as opool:
            o2 = opool.tile([64, np_pad], F32)
            emit_aggregation(tc, nc, plan, ptab[:min(SPLIT, n_table_rows), :],
                             ptab[hb:, :], idx_d, sw_d, o2, 64)
            nc.sync.dma_start(out=o2_d[:], in_=o2[:])
    nc.compile()
    return nc


# ---------------------------------------------------------------------------
# Runners
# ---------------------------------------------------------------------------

def sim_runner(nc, in_maps):
    from concourse.bass_interp import CoreSim
    outs = []
    for m in in_maps:
        sim = CoreSim(nc, trace=False, require_finite=False, require_nnan=False)
        for name, val in m.items():
            sim.tensor(name)[:] = val
        sim.simulate(check_with_hw=False)
        out = {}
        for alloc in nc.m.functions[0].allocations:
            if isinstance(alloc, mybir.MemoryLocationSet) and alloc.kind == "ExternalOutput":
                name = alloc.memorylocations[0].name
                out[name] = np.array(sim.tensor(name))
        outs.append(out)
    return outs


def _install_ntff_hook():
    """The agent image's antenv lacks axon_hooks; synthesize it so
    run_bass_kernel_spmd(trace=True) can NTFF-profile via the axon .so."""
    import types
    if "antenv.axon_hooks" in sys.modules:
        return True
    try:
        from trn_agent_boot.trn_boot import _ntff_profile_via_ctypes
        hook = _ntff_profile_via_ctypes("/opt/axon/libaxon_pjrt.so")
    except Exception:
        return False
    mod = types.ModuleType("antenv.axon_hooks")
    mod._hook = hook
    mod.set_axon_ntff_profile_hook = lambda h: setattr(mod, "_hook", h)
    mod.get_axon_ntff_profile_hook = lambda: mod._hook
    sys.modules["antenv.axon_hooks"] = mod
    try:
        import antenv
        antenv.axon_hooks = mod
    except Exception:
        pass
    return True


def hw_runner_factory(trace=False, label=""):
    from concourse.bass_utils import run_bass_kernel_spmd
    if trace:
        trace = _install_ntff_hook()
    times = {}

    def hw_runner(nc, in_maps):
        res = run_bass_kernel_spmd(nc, in_maps, core_ids=list(range(len(in_maps))),
                                   trace=trace)
        times[label or "t"] = times.get(label or "t", 0) + (res.exec_time_ns or 0)
        hw_runner.last = res
        return res.results

    hw_runner.times = times
    return hw_runner


# ---------------------------------------------------------------------------
# Full host orchestration
# ---------------------------------------------------------------------------

def run(feature, edge_weight, layer1, layer2, src, dst, mask1, mask2,
        n_cores=8, runner=None, trace=False):
    """runner(nc, in_maps) -> list of out dicts; defaults to HW spmd."""
    N = feature.shape[0]
    E = src.shape[0]
    T = mask1.shape[0]
    npc = -(-N // n_cores)          # nodes per core
    nrows = ((N + 127) // 128) * 128
    src = np.asarray(src).astype(np.int64)
    dst = np.asarray(dst).astype(np.int64)
    w = np.asarray(edge_weight).astype(np.float32)

    core_of = dst // npc
    per_core = []
    for k in range(n_cores):
        m = core_of == k
        per_core.append((src[m], dst[m] - k * npc, w[m]))

    real_max = merge_counts([count_core(s, d, npc) for (s, d, _) in per_core])
    plan = Plan(npc, real_max)

    idx_all, sw_all = [], []
    for k in range(n_cores):
        s, d, ww = per_core[k]
        idx_np, sw_np = build_core_data(plan, s, d, ww)
        idx_all.append(idx_np)
        sw_all.append(sw_np)

    # feature table bf16 [nrows, 128]
    ftab = np.zeros((nrows, 128), ml_dtypes.bfloat16)
    ftab[:N] = feature.astype(ml_dtypes.bfloat16)

    # premasked weights
    l1m = (np.asarray(layer1)[None] * np.asarray(mask1)).astype(np.float32)
    l2m = np.zeros((T, 128, 32), np.float32)
    l2m[:, :, :16] = np.asarray(layer2)[None] * np.asarray(mask2)

    nc_a = build_launch_a(plan, nrows)
    in_maps_a = [
        {"ftab": ftab, "idx": idx_all[k], "sw": sw_all[k], "l1": l1m, "l2": l2m}
        for k in range(n_cores)
    ]
    res_a = runner(nc_a, in_maps_a)

    # assemble p-table: rows n -> 64 p values (r = 32t + o from pt rows)
    np_pad = plan.ngroups * GROUP_W * W
    ptab = np.zeros((nrows, 128), ml_dtypes.bfloat16)
    for k in range(n_cores):
        pt = res_a[k]["pt"]  # [128, np_pad]
        rows = np.concatenate([pt[32 * t:32 * t + 16] for t in range(T)])  # [64, NP]
        n0, n1 = k * npc, min((k + 1) * npc, N)
        ptab[n0:n1, :64] = rows[:, : n1 - n0].T.astype(ml_dtypes.bfloat16)

    nc_b = build_launch_b(plan, nrows)
    in_maps_b = [
        {"ptab": ptab, "idx": idx_all[k], "sw": sw_all[k]}
        for k in range(n_cores)
    ]
    res_b = runner(nc_b, in_maps_b)

    out = np.zeros((T, N, 16), np.float32)
    for k in range(n_cores):
        o2 = res_b[k]["o2"]  # [64, np_pad]
        n0, n1 = k * npc, min((k + 1) * npc, N)
        blk = o2[:, : n1 - n0].reshape(T, 16, n1 - n0)
        out[:, n0:n1, :] = blk.transpose(0, 2, 1)
    return out


# ---------------------------------------------------------------------------
# Harness entry point
# ---------------------------------------------------------------------------

def kernel(feature, edge_weight, layer1, layer2, src, dst, mask1, mask2):
    """Full (unsharded) inputs -> full [T, N, 16] float32 output.

    Shards edges by dst range across 8 NeuronCores, runs two Bass launches
    (aggregation-1 + GEMMs, then aggregation-2), gathers on host.
    """
    import os
    trace = bool(os.environ.get("KERNEL_TRACE"))
    runner = hw_runner_factory(trace=trace)
    out = run(
        np.asarray(feature, np.float32),
        np.asarray(edge_weight, np.float32),
        np.asarray(layer1, np.float32),
        np.asarray(layer2, np.float32),
        np.asarray(src),
        np.asarray(dst),
        np.asarray(mask1),
        np.asarray(mask2),
        n_cores=8,
        runner=runner,
    )
    kernel.exec_time_ns = sum(runner.times.values()) if trace else None
    return out


# revision 15
# speedup vs baseline: 1.0276x; 1.0064x over previous
"""GNN message-passing Bass kernel for TRN2 (8 cores, SPMD).

Math (reference):
  h0 = segsum_dst(w_e * feature[src_e])              # [N, 128]
  for t in 0..3:
    h  = relu(h0 @ (layer1*mask1[t]))                # [N, 128]
    p_t = h @ (layer2*mask2[t])                      # [N, 16]
  out_t = segsum_dst(w_e * p_t[src_e])               # [N, 16]  (A @ p_t)

Key transformation: out_t = A @ (h_t @ W2_t) so the second aggregation runs on
16-wide vectors (64 for all t stacked), not 128-wide.

Implementation: two launches.
  Launch A: edge-gather from bf16 feature table (HBM), scatter via per-tile
    matmul  h0T[f, win] += M_tile.T @ S'_tile  (feature-major accumulation in
    PSUM), then the dense GEMMs (fp32) -> pT staged [128, NP] (rows 32t+o),
    fused per 512-column group so the GEMM overlaps later groups' gathers.
  Host: assemble p-table [50176, 128] bf16 (64 values + 64 zero pad per row).
  Launch B: same aggregation structure against the p-table -> out2T [64, NP].

Edges are partitioned by dst across cores (6250 nodes each); each tile of 128
edges belongs to one 64-node dst window and one src bucket (src < 32768 or
not, because gather indices are int16). The SWDGE gather (extended-inst
DMAGatherAnt on the Pool engine) is the kernel bottleneck at ~8.5ns per
128-token chunk-slot of descriptor generation; invalid lanes still emit dummy
descriptors, so the floor is the tile count and all pad slots are plain valid
tokens (index 0, weight 0). Tile counts per (group, window, bucket) are padded
to the max across cores so one SPMD program serves all 8.
"""

import sys

sys.path.insert(0, "/opt/trn_rl_repo")

import numpy as np
import ml_dtypes

import concourse.bass as bass
import concourse.bacc as bacc
import concourse.mybir as mybir
import concourse.tile as tile

F32 = mybir.dt.float32
BF16 = mybir.dt.bfloat16
I16 = mybir.dt.int16

TILE = 128          # edges per tile
W = 512             # dst nodes per window (matmul moving width)
GROUP_W = 1         # windows per psum group (1*512 = 512 fp32 cols = 1 bank)
OP_TILES = 8        # max tiles per dma_gather op (1024 tokens; >=1536 tokens
                    # per op overflows the SWDGE descriptor ring on HW)
SPLIT = 32768       # int16 index split
GBUFS = 8           # gather-buffer rotation depth


# ---------------------------------------------------------------------------
# Host-side planning
# ---------------------------------------------------------------------------

class Plan:
    """Uniform (cross-core) tile plan for one aggregation.

    real_max: [ngroups][2 buckets][GROUP_W windows] -> max (over cores) real
    token count in that (window, bucket) segment.

    Tiles are packed per (group, bucket) run with uniform per-window slot
    offsets (cross-core max counts), so tiles may span window boundaries;
    a tile issues one matmul per window it overlaps. This removes the
    per-window ceil-to-128 padding that costs SWDGE descriptor-generation
    time (the kernel bottleneck).
    """

    def __init__(self, n_nodes, real_max):
        self.n_nodes = n_nodes
        self.nwin = -(-n_nodes // W)
        self.n_nodes_pad = self.nwin * W
        self.ngroups = -(-self.nwin // GROUP_W)
        self.real_max = real_max
        self.tile_bucket = []
        self.groups = []     # per group: dict(ops=[...], c0, c1)
        self.run_info = {}   # (g, b) -> (slot_base, offs[GROUP_W], cnts[GROUP_W])
        c = 0
        nblk = 0
        for g in range(self.ngroups):
            c0g = c
            ops = []
            mms_group = []   # (tile c, wl, blk)
            for b in range(2):
                cnts = [int(real_max[g][b][wi]) for wi in range(GROUP_W)]
                total = sum(cnts)
                if total == 0:
                    continue
                ntiles = -(-total // TILE)
                run_c0 = c
                offs = [0] * GROUP_W
                for wi in range(1, GROUP_W):
                    offs[wi] = offs[wi - 1] + cnts[wi - 1]
                self.run_info[(g, b)] = (run_c0 * TILE, offs, cnts)
                tile_mms = []
                for t in range(ntiles):
                    s0, s1 = t * TILE, (t + 1) * TILE
                    mms = []
                    for wi in range(GROUP_W):
                        if cnts[wi] == 0:
                            continue
                        if offs[wi] < s1 and offs[wi] + cnts[wi] > s0:
                            mms.append((run_c0 + t, wi, nblk))
                            nblk += 1
                    tile_mms.append(mms)
                    self.tile_bucket.append(b)
                c += ntiles
                # chunk the run into gather ops
                i = 0
                while i < ntiles:
                    n = min(OP_TILES, ntiles - i)
                    op_mms = [m for tm in tile_mms[i:i + n] for m in tm]
                    blk0 = op_mms[0][2]
                    ops.append({
                        "c0": run_c0 + i, "n": n,
                        "blk0": blk0, "nblk": len(op_mms),
                        "mms": [(tc - (run_c0 + i), wl, bk - blk0)
                                for (tc, wl, bk) in op_mms],
                    })
                    mms_group.extend(op_mms)
                    i += n
            assert mms_group, f"group {g} has no matmuls"
            self.groups.append({"ops": ops, "c0": c0g, "c1": c,
                                "n_mms": len(mms_group)})
        self.nt = c
        self.nblk = nblk


def count_core(srct, dstloc, n_nodes):
    """Per-core real token counts [ngroups][2][GROUP_W]."""
    nwin = -(-n_nodes // W)
    ngroups = -(-nwin // GROUP_W)
    win = dstloc // W
    bucket = (srct >= SPLIT).astype(np.int64)
    cnt = np.zeros((ngroups, 2, GROUP_W), np.int64)
    key = (win * 2 + bucket).astype(np.int64)
    bc = np.bincount(key, minlength=nwin * 2)
    for gw in range(nwin):
        g, wi = divmod(gw, GROUP_W)
        for b in range(2):
            cnt[g][b][wi] = bc[gw * 2 + b]
    return cnt


def merge_counts(all_counts):
    return np.maximum.reduce(all_counts)


def build_core_data(plan: Plan, srct, dstloc, wgt):
    """Per-core idx + scatter-weight arrays matching the uniform plan.

    Every slot is a valid token (pads gather row 0 with weight 0): invalid
    lanes would still cost dummy descriptors, and num_idxs must be uniform
    across cores to keep the NX descriptor-ring accounting in sync.
    Returns idx_np [128, NT*8] int16, sw_np [128, NBLK*W] bf16.
    """
    nt, nblk = plan.nt, plan.nblk
    tok_flat = np.zeros(nt * TILE, np.int64)
    sw = np.zeros((TILE, nblk * W), np.float32)

    win = dstloc // W
    bucket = (srct >= SPLIT).astype(np.int64)
    order = np.lexsort((srct, win * 2 + bucket))
    s_srct = srct[order]
    s_dstloc = dstloc[order]
    s_w = wgt[order].astype(np.float32)
    s_key = (win * 2 + bucket)[order]

    bounds = np.flatnonzero(np.r_[True, s_key[1:] != s_key[:-1], True])
    seg = {}
    for a, b in zip(bounds[:-1], bounds[1:]):
        seg[int(s_key[a])] = (int(a), int(b))

    # (tile c, window wl) -> sw block id
    blk_of = {}
    for grp in plan.groups:
        for op in grp["ops"]:
            for (i, wl, bl) in op["mms"]:
                blk_of[(op["c0"] + i, wl)] = op["blk0"] + bl

    for (g, b), (slot_base, offs, cnts) in plan.run_info.items():
        for wi in range(GROUP_W):
            if cnts[wi] == 0:
                continue
            gw = g * GROUP_W + wi
            a, e = seg.get(gw * 2 + b, (0, 0))
            n = e - a
            assert n <= cnts[wi], f"plan too small for seg {(gw, b)}"
            if n == 0:
                continue
            s0 = slot_base + offs[wi]
            tok_flat[s0:s0 + n] = s_srct[a:e] - b * SPLIT
            slots = np.arange(s0, s0 + n)
            p = slots % TILE
            tc = slots // TILE
            blks = np.array([blk_of[(c, wi)] for c in np.unique(tc)])
            blk_per_slot = np.array([blk_of[(int(c), wi)] for c in tc])
            cols = s_dstloc[a:e] - gw * W
            sw[p, blk_per_slot * W + cols] = s_w[a:e]

    ni = tok_flat.shape[0]
    idx_np = np.tile(tok_flat.reshape(ni // 16, 16).T, (8, 1)).astype(np.int16)
    sw_np = np.ascontiguousarray(sw).astype(ml_dtypes.bfloat16)
    return idx_np, sw_np


# ---------------------------------------------------------------------------
# Device-side emit
# ---------------------------------------------------------------------------

def emit_aggregation(tc, nc, plan: Plan, table_lo, table_hi, idx_dram, sw_dram,
                     out_sbuf, out_rows, elem=128, per_group=None):
    """Gather + matmul-scatter. out_sbuf [>=out_rows, ngroups*512] fp32."""
    MAXBLK = OP_TILES + GROUP_W - 1
    with (
        tc.tile_pool(name="agg_idx", bufs=1) as ipool,
        tc.tile_pool(name="agg_g", bufs=GBUFS) as gpool,
        tc.tile_pool(name="agg_s", bufs=8) as spool,
        tc.tile_pool(name="agg_ps", bufs=3, space="PSUM") as pspool,
    ):
        ni = plan.nt * TILE
        idx_t = ipool.tile([128, ni // 16], I16)
        quarter = (ni // 16) // 4
        nc.sync.dma_start(out=idx_t[:, :quarter], in_=idx_dram[:, :quarter])
        nc.sync.dma_start(out=idx_t[:, quarter:], in_=idx_dram[:, quarter:])
        for g, grp in enumerate(plan.groups):
            ps = pspool.tile([128, GROUP_W * W], F32)
            mm_seen = 0
            for op in grp["ops"]:
                c0, n, blk0, nblk = op["c0"], op["n"], op["blk0"], op["nblk"]
                assert nblk <= MAXBLK
                b = plan.tile_bucket[c0]
                gd = gpool.tile([128, OP_TILES, elem], BF16)
                swt = spool.tile([128, MAXBLK * W], BF16)
                nc.sync.dma_start(
                    out=swt[:, : nblk * W],
                    in_=sw_dram[:, blk0 * W:(blk0 + nblk) * W],
                )
                nc.gpsimd.dma_gather(
                    out_ap=gd[:, :n, :],
                    in_ap=(table_hi if b else table_lo),
                    idxs_ap=idx_t[:, c0 * 8:(c0 + n) * 8],
                    num_idxs=n * TILE,
                    num_idxs_reg=n * TILE,
                    elem_size=elem,
                )
                for (i, wl, bl) in op["mms"]:
                    mm_seen += 1
                    nc.tensor.matmul(
                        out=ps[:, wl * W:(wl + 1) * W],
                        lhsT=gd[:, i, :],
                        rhs=swt[:, bl * W:(bl + 1) * W],
                        start=(mm_seen == 1),
                        stop=(mm_seen == grp["n_mms"]),
                    )
            nc.vector.tensor_copy(
                out=out_sbuf[:out_rows, g * GROUP_W * W:(g + 1) * GROUP_W * W],
                in_=ps[:out_rows, :],
            )
            if per_group is not None:
                per_group(g)


def build_launch_a(plan: Plan, n_table_rows):
    """Launch A: aggregation-1 + GEMMs -> pt [128, NP] (rows 32t+o used)."""
    np_pad = plan.ngroups * GROUP_W * W
    nc = bacc.Bacc("TRN2", target_bir_lowering=False, debug=False, num_devices=8)
    ftab = nc.dram_tensor("ftab", [n_table_rows, 128], BF16, kind="ExternalInput")
    idx_d = nc.dram_tensor("idx", [128, plan.nt * 8], I16, kind="ExternalInput")
    sw_d = nc.dram_tensor("sw", [128, plan.nblk * W], BF16, kind="ExternalInput")
    l1_d = nc.dram_tensor("l1", [4, 128, 128], F32, kind="ExternalInput")  # premasked
    l2_d = nc.dram_tensor("l2", [4, 128, 32], F32, kind="ExternalInput")   # premasked+padded
    pt_d = nc.dram_tensor("pt", [128, np_pad], F32, kind="ExternalOutput")

    hb = SPLIT if n_table_rows > SPLIT else 0
    with tile.TileContext(nc) as tc:
        with (
            tc.tile_pool(name="h0", bufs=1) as h0pool,
            tc.tile_pool(name="wts", bufs=1) as wpool,
            tc.tile_pool(name="hs", bufs=3) as hspool,
            tc.tile_pool(name="ptst", bufs=1) as ptpool,
            tc.tile_pool(name="ps1", bufs=2, space="PSUM") as ps1pool,
            tc.tile_pool(name="ps2", bufs=2, space="PSUM") as ps2pool,
        ):
            h0T = h0pool.tile([128, np_pad], F32)
            w1 = wpool.tile([128, 4, 128], F32)
            nc.sync.dma_start(out=w1[:], in_=l1_d.rearrange("t k h -> k t h"))
            w2 = wpool.tile([128, 4, 32], F32)
            nc.sync.dma_start(out=w2[:], in_=l2_d.rearrange("t k h -> k t h"))
            ptst = ptpool.tile([128, np_pad], F32)

            def per_group(ch):
                sl = slice(ch * 512, (ch + 1) * 512)
                ps2 = ps2pool.tile([128, 512], F32)
                for t in range(4):
                    ps1 = ps1pool.tile([128, 512], F32)
                    nc.tensor.matmul(out=ps1[:], lhsT=w1[:, t, :], rhs=h0T[:, sl],
                                     start=True, stop=True)
                    hs = hspool.tile([128, 512], F32)
                    nc.scalar.activation(
                        out=hs[:], in_=ps1[:],
                        func=mybir.ActivationFunctionType.Relu,
                    )
                    nc.tensor.matmul(out=ps2[32 * t:32 * t + 32, :],
                                     lhsT=w2[:, t, :], rhs=hs[:],
                                     start=True, stop=True,
                                     tile_position=(0, 32 * t))
                nc.vector.tensor_copy(out=ptst[:, sl], in_=ps2[:])
                nc.sync.dma_start(out=pt_d[:, sl], in_=ptst[:, sl])

            emit_aggregation(tc, nc, plan, ftab[:min(SPLIT, n_table_rows), :],
                             ftab[hb:, :], idx_d, sw_d, h0T, 128,
                             per_group=per_group)
    nc.compile()
    return nc


def build_launch_b(plan: Plan, n_table_rows):
    """Launch B: aggregation-2 against p-table -> o2 [64, NP]."""
    np_pad = plan.ngroups * GROUP_W * W
    nc = bacc.Bacc("TRN2", target_bir_lowering=False, debug=False, num_devices=8)
    ptab = nc.dram_tensor("ptab", [n_table_rows, 128], BF16, kind="ExternalInput")
    idx_d = nc.dram_tensor("idx", [128, plan.nt * 8], I16, kind="ExternalInput")
    sw_d = nc.dram_tensor("sw", [128, plan.nblk * W], BF16, kind="ExternalInput")
    o2_d = nc.dram_tensor("o2", [64, np_pad], F32, kind="ExternalOutput")

    hb = SPLIT if n_table_rows > SPLIT else 0
    with tile.TileContext(nc) as tc:
        with tc.tile_pool(name="o2", bufs=1) as opool:
            o2 = opool.tile([64, np_pad], F32)
            emit_aggregation(tc, nc, plan, ptab[:min(SPLIT, n_table_rows), :],
                             ptab[hb:, :], idx_d, sw_d, o2, 64)
            nc.sync.dma_start(out=o2_d[:], in_=o2[:])
    nc.compile()
    return nc


# ---------------------------------------------------------------------------
# Runners
# ---------------------------------------------------------------------------

def sim_runner(nc, in_maps):
    from concourse.bass_interp import CoreSim
    outs = []
    for m in in_maps:
        sim = CoreSim(nc, trace=False, require_finite=False, require_nnan=False)
        for name, val in m.items():
            sim.tensor(name)[:] = val
        sim.simulate(check_with_hw=False)
        out = {}
        for alloc in nc.m.functions[0].allocations:
            if isinstance(alloc, mybir.MemoryLocationSet) and alloc.kind == "ExternalOutput":
                name = alloc.memorylocations[0].name
                out[name] = np.array(sim.tensor(name))
        outs.append(out)
    return outs


def _install_ntff_hook():
    """The agent image's antenv lacks axon_hooks; synthesize it so
    run_bass_kernel_spmd(trace=True) can NTFF-profile via the axon .so."""
    import types
    if "antenv.axon_hooks" in sys.modules:
        return True
    try:
        from trn_agent_boot.trn_boot import _ntff_profile_via_ctypes
        hook = _ntff_profile_via_ctypes("/opt/axon/libaxon_pjrt.so")
    except Exception:
        return False
    mod = types.ModuleType("antenv.axon_hooks")
    mod._hook = hook
    mod.set_axon_ntff_profile_hook = lambda h: setattr(mod, "_hook", h)
    mod.get_axon_ntff_profile_hook = lambda: mod._hook
    sys.modules["antenv.axon_hooks"] = mod
    try:
        import antenv
        antenv.axon_hooks = mod
    except Exception:
        pass
    return True


def hw_runner_factory(trace=False, label=""):
    from concourse.bass_utils import run_bass_kernel_spmd
    if trace:
        trace = _install_ntff_hook()
    times = {}

    def hw_runner(nc, in_maps):
        res = run_bass_kernel_spmd(nc, in_maps, core_ids=list(range(len(in_maps))),
                                   trace=trace)
        times[label or "t"] = times.get(label or "t", 0) + (res.exec_time_ns or 0)
        hw_runner.last = res
        return res.results

    hw_runner.times = times
    return hw_runner


# ---------------------------------------------------------------------------
# Full host orchestration
# ---------------------------------------------------------------------------

def run(feature, edge_weight, layer1, layer2, src, dst, mask1, mask2,
        n_cores=8, runner=None, trace=False):
    """runner(nc, in_maps) -> list of out dicts; defaults to HW spmd."""
    N = feature.shape[0]
    E = src.shape[0]
    T = mask1.shape[0]
    npc = -(-N // n_cores)          # nodes per core
    nrows = ((N + 127) // 128) * 128
    src = np.asarray(src).astype(np.int64)
    dst = np.asarray(dst).astype(np.int64)
    w = np.asarray(edge_weight).astype(np.float32)

    core_of = dst // npc
    per_core = []
    for k in range(n_cores):
        m = core_of == k
        per_core.append((src[m], dst[m] - k * npc, w[m]))

    real_max = merge_counts([count_core(s, d, npc) for (s, d, _) in per_core])
    plan = Plan(npc, real_max)

    idx_all, sw_all = [], []
    for k in range(n_cores):
        s, d, ww = per_core[k]
        idx_np, sw_np = build_core_data(plan, s, d, ww)
        idx_all.append(idx_np)
        sw_all.append(sw_np)

    # feature table bf16 [nrows, 128]
    ftab = np.zeros((nrows, 128), ml_dtypes.bfloat16)
    ftab[:N] = feature.astype(ml_dtypes.bfloat16)

    # premasked weights
    l1m = (np.asarray(layer1)[None] * np.asarray(mask1)).astype(np.float32)
    l2m = np.zeros((T, 128, 32), np.float32)
    l2m[:, :, :16] = np.asarray(layer2)[None] * np.asarray(mask2)

    nc_a = build_launch_a(plan, nrows)
    in_maps_a = [
        {"ftab": ftab, "idx": idx_all[k], "sw": sw_all[k], "l1": l1m, "l2": l2m}
        for k in range(n_cores)
    ]
    res_a = runner(nc_a, in_maps_a)

    # assemble p-table: rows n -> 64 p values (r = 32t + o from pt rows)
    np_pad = plan.ngroups * GROUP_W * W
    ptab = np.zeros((nrows, 128), ml_dtypes.bfloat16)
    for k in range(n_cores):
        pt = res_a[k]["pt"]  # [128, np_pad]
        rows = np.concatenate([pt[32 * t:32 * t + 16] for t in range(T)])  # [64, NP]
        n0, n1 = k * npc, min((k + 1) * npc, N)
        ptab[n0:n1, :64] = rows[:, : n1 - n0].T.astype(ml_dtypes.bfloat16)

    nc_b = build_launch_b(plan, nrows)
    in_maps_b = [
        {"ptab": ptab, "idx": idx_all[k], "sw": sw_all[k]}
        for k in range(n_cores)
    ]
    res_b = runner(nc_b, in_maps_b)

    out = np.zeros((T, N, 16), np.float32)
    for k in range(n_cores):
        o2 = res_b[k]["o2"]  # [64, np_pad]
        n0, n1 = k * npc, min((k + 1) * npc, N)
        blk = o2[:, : n1 - n0].reshape(T, 16, n1 - n0)
        out[:, n0:n1, :] = blk.transpose(0, 2, 1)
    return out


# ---------------------------------------------------------------------------
# Harness entry point
# ---------------------------------------------------------------------------

def kernel(feature, edge_weight, layer1, layer2, src, dst, mask1, mask2):
    """Full (unsharded) inputs -> full [T, N, 16] float32 output.

    Shards edges by dst range across 8 NeuronCores, runs two Bass launches
    (aggregation-1 + GEMMs, then aggregation-2), gathers on host.
    """
    import os
    trace = bool(os.environ.get("KERNEL_TRACE"))
    runner = hw_runner_factory(trace=trace)
    out = run(
        np.asarray(feature, np.float32),
        np.asarray(edge_weight, np.float32),
        np.asarray(layer1, np.float32),
        np.asarray(layer2, np.float32),
        np.asarray(src),
        np.asarray(dst),
        np.asarray(mask1),
        np.asarray(mask2),
        n_cores=8,
        runner=runner,
    )
    kernel.exec_time_ns = sum(runner.times.values()) if trace else None
    return out


# revision 16
# speedup vs baseline: 1.0298x; 1.0021x over previous
"""GNN message-passing Bass kernel for TRN2 (8 cores, SPMD).

Math (reference):
  h0 = segsum_dst(w_e * feature[src_e])              # [N, 128]
  for t in 0..3:
    h  = relu(h0 @ (layer1*mask1[t]))                # [N, 128]
    p_t = h @ (layer2*mask2[t])                      # [N, 16]
  out_t = segsum_dst(w_e * p_t[src_e])               # [N, 16]  (A @ p_t)

Key transformation: out_t = A @ (h_t @ W2_t) so the second aggregation runs on
16-wide vectors (64 for all t stacked), not 128-wide.

Implementation: two launches.
  Launch A: edge-gather from bf16 feature table (HBM), scatter via per-tile
    matmul  h0T[f, win] += M_tile.T @ S'_tile  (feature-major accumulation in
    PSUM), then the dense GEMMs (fp32) -> pT staged [128, NP] (rows 32t+o),
    fused per 512-column group so the GEMM overlaps later groups' gathers.
  Host: assemble p-table [50176, 128] bf16 (64 values + 64 zero pad per row).
  Launch B: same aggregation structure against the p-table -> out2T [64, NP].

Edges are partitioned by dst across cores (6250 nodes each); each tile of 128
edges belongs to one 64-node dst window and one src bucket (src < 32768 or
not, because gather indices are int16). The SWDGE gather (extended-inst
DMAGatherAnt on the Pool engine) is the kernel bottleneck at ~8.5ns per
128-token chunk-slot of descriptor generation; invalid lanes still emit dummy
descriptors, so the floor is the tile count and all pad slots are plain valid
tokens (index 0, weight 0). Tile counts per (group, window, bucket) are padded
to the max across cores so one SPMD program serves all 8.
"""

import sys

sys.path.insert(0, "/opt/trn_rl_repo")

import numpy as np
import ml_dtypes

import concourse.bass as bass
import concourse.bacc as bacc
import concourse.mybir as mybir
import concourse.tile as tile

F32 = mybir.dt.float32
BF16 = mybir.dt.bfloat16
I16 = mybir.dt.int16

TILE = 128          # edges per tile
W = 512             # dst nodes per window (matmul moving width)
GROUP_W = 1         # windows per psum group (1*512 = 512 fp32 cols = 1 bank)
OP_TILES = 8        # max tiles per dma_gather op (1024 tokens; >=1536 tokens
                    # per op overflows the SWDGE descriptor ring on HW)
SPLIT = 32768       # int16 index split
GBUFS = 8           # gather-buffer rotation depth


# ---------------------------------------------------------------------------
# Host-side planning
# ---------------------------------------------------------------------------

class Plan:
    """Uniform (cross-core) tile plan for one aggregation.

    real_max: [ngroups][2 buckets][GROUP_W windows] -> max (over cores) real
    token count in that (window, bucket) segment.

    Tiles are packed per (group, bucket) run with uniform per-window slot
    offsets (cross-core max counts), so tiles may span window boundaries;
    a tile issues one matmul per window it overlaps. This removes the
    per-window ceil-to-128 padding that costs SWDGE descriptor-generation
    time (the kernel bottleneck).
    """

    def __init__(self, n_nodes, real_max):
        self.n_nodes = n_nodes
        self.nwin = -(-n_nodes // W)
        self.n_nodes_pad = self.nwin * W
        self.ngroups = -(-self.nwin // GROUP_W)
        self.real_max = real_max
        self.tile_bucket = []
        self.groups = []     # per group: dict(ops=[...], c0, c1)
        self.run_info = {}   # (g, b) -> (slot_base, offs[GROUP_W], cnts[GROUP_W])
        c = 0
        nblk = 0
        for g in range(self.ngroups):
            c0g = c
            ops = []
            mms_group = []   # (tile c, wl, blk)
            for b in range(2):
                cnts = [int(real_max[g][b][wi]) for wi in range(GROUP_W)]
                total = sum(cnts)
                if total == 0:
                    continue
                ntiles = -(-total // TILE)
                run_c0 = c
                offs = [0] * GROUP_W
                for wi in range(1, GROUP_W):
                    offs[wi] = offs[wi - 1] + cnts[wi - 1]
                self.run_info[(g, b)] = (run_c0 * TILE, offs, cnts)
                tile_mms = []
                for t in range(ntiles):
                    s0, s1 = t * TILE, (t + 1) * TILE
                    mms = []
                    for wi in range(GROUP_W):
                        if cnts[wi] == 0:
                            continue
                        if offs[wi] < s1 and offs[wi] + cnts[wi] > s0:
                            mms.append((run_c0 + t, wi, nblk))
                            nblk += 1
                    tile_mms.append(mms)
                    self.tile_bucket.append(b)
                c += ntiles
                # chunk the run into gather ops
                i = 0
                while i < ntiles:
                    n = min(OP_TILES, ntiles - i)
                    op_mms = [m for tm in tile_mms[i:i + n] for m in tm]
                    blk0 = op_mms[0][2]
                    ops.append({
                        "c0": run_c0 + i, "n": n,
                        "blk0": blk0, "nblk": len(op_mms),
                        "mms": [(tc - (run_c0 + i), wl, bk - blk0)
                                for (tc, wl, bk) in op_mms],
                    })
                    mms_group.extend(op_mms)
                    i += n
            assert mms_group, f"group {g} has no matmuls"
            self.groups.append({"ops": ops, "c0": c0g, "c1": c,
                                "n_mms": len(mms_group)})
        self.nt = c
        self.nblk = nblk


def count_core(srct, dstloc, n_nodes):
    """Per-core real token counts [ngroups][2][GROUP_W]."""
    nwin = -(-n_nodes // W)
    ngroups = -(-nwin // GROUP_W)
    win = dstloc // W
    bucket = (srct >= SPLIT).astype(np.int64)
    cnt = np.zeros((ngroups, 2, GROUP_W), np.int64)
    key = (win * 2 + bucket).astype(np.int64)
    bc = np.bincount(key, minlength=nwin * 2)
    for gw in range(nwin):
        g, wi = divmod(gw, GROUP_W)
        for b in range(2):
            cnt[g][b][wi] = bc[gw * 2 + b]
    return cnt


def merge_counts(all_counts):
    return np.maximum.reduce(all_counts)


def build_core_data(plan: Plan, srct, dstloc, wgt):
    """Per-core idx + scatter-weight arrays matching the uniform plan.

    Every slot is a valid token (pads gather row 0 with weight 0): invalid
    lanes would still cost dummy descriptors, and num_idxs must be uniform
    across cores to keep the NX descriptor-ring accounting in sync.
    Returns idx_np [128, NT*8] int16, sw_np [128, NBLK*W] bf16.
    """
    nt, nblk = plan.nt, plan.nblk
    tok_flat = np.zeros(nt * TILE, np.int64)
    sw = np.zeros((TILE, nblk * W), np.float32)

    win = dstloc // W
    bucket = (srct >= SPLIT).astype(np.int64)
    order = np.lexsort((srct, win * 2 + bucket))
    s_srct = srct[order]
    s_dstloc = dstloc[order]
    s_w = wgt[order].astype(np.float32)
    s_key = (win * 2 + bucket)[order]

    bounds = np.flatnonzero(np.r_[True, s_key[1:] != s_key[:-1], True])
    seg = {}
    for a, b in zip(bounds[:-1], bounds[1:]):
        seg[int(s_key[a])] = (int(a), int(b))

    # (tile c, window wl) -> sw block id
    blk_of = {}
    for grp in plan.groups:
        for op in grp["ops"]:
            for (i, wl, bl) in op["mms"]:
                blk_of[(op["c0"] + i, wl)] = op["blk0"] + bl

    for (g, b), (slot_base, offs, cnts) in plan.run_info.items():
        for wi in range(GROUP_W):
            if cnts[wi] == 0:
                continue
            gw = g * GROUP_W + wi
            a, e = seg.get(gw * 2 + b, (0, 0))
            n = e - a
            assert n <= cnts[wi], f"plan too small for seg {(gw, b)}"
            if n == 0:
                continue
            s0 = slot_base + offs[wi]
            tok_flat[s0:s0 + n] = s_srct[a:e] - b * SPLIT
            slots = np.arange(s0, s0 + n)
            p = slots % TILE
            tc = slots // TILE
            blks = np.array([blk_of[(c, wi)] for c in np.unique(tc)])
            blk_per_slot = np.array([blk_of[(int(c), wi)] for c in tc])
            cols = s_dstloc[a:e] - gw * W
            sw[p, blk_per_slot * W + cols] = s_w[a:e]

    ni = tok_flat.shape[0]
    idx_np = np.tile(tok_flat.reshape(ni // 16, 16).T, (8, 1)).astype(np.int16)
    sw_np = np.ascontiguousarray(sw).astype(ml_dtypes.bfloat16)
    return idx_np, sw_np


# ---------------------------------------------------------------------------
# Device-side emit
# ---------------------------------------------------------------------------

def emit_aggregation(tc, nc, plan: Plan, table_lo, table_hi, idx_dram, sw_dram,
                     out_sbuf, out_rows, elem=128, per_group=None):
    """Gather + matmul-scatter. out_sbuf [>=out_rows, ngroups*512] fp32."""
    MAXBLK = OP_TILES + GROUP_W - 1
    with (
        tc.tile_pool(name="agg_idx", bufs=1) as ipool,
        tc.tile_pool(name="agg_g", bufs=GBUFS) as gpool,
        tc.tile_pool(name="agg_s", bufs=8) as spool,
        tc.tile_pool(name="agg_ps", bufs=3, space="PSUM") as pspool,
    ):
        ni = plan.nt * TILE
        idx_t = ipool.tile([128, ni // 16], I16)
        quarter = (ni // 16) // 4
        nc.sync.dma_start(out=idx_t[:, :quarter], in_=idx_dram[:, :quarter])
        nc.sync.dma_start(out=idx_t[:, quarter:], in_=idx_dram[:, quarter:])
        for g, grp in enumerate(plan.groups):
            ps = pspool.tile([128, GROUP_W * W], F32)
            mm_seen = 0
            for op in grp["ops"]:
                c0, n, blk0, nblk = op["c0"], op["n"], op["blk0"], op["nblk"]
                assert nblk <= MAXBLK
                b = plan.tile_bucket[c0]
                gd = gpool.tile([128, OP_TILES, elem], BF16)
                swt = spool.tile([128, MAXBLK * W], BF16)
                nc.sync.dma_start(
                    out=swt[:, : nblk * W],
                    in_=sw_dram[:, blk0 * W:(blk0 + nblk) * W],
                )
                nc.gpsimd.dma_gather(
                    out_ap=gd[:, :n, :],
                    in_ap=(table_hi if b else table_lo),
                    idxs_ap=idx_t[:, c0 * 8:(c0 + n) * 8],
                    num_idxs=n * TILE,
                    num_idxs_reg=n * TILE,
                    elem_size=elem,
                )
                for (i, wl, bl) in op["mms"]:
                    mm_seen += 1
                    nc.tensor.matmul(
                        out=ps[:, wl * W:(wl + 1) * W],
                        lhsT=gd[:, i, :],
                        rhs=swt[:, bl * W:(bl + 1) * W],
                        start=(mm_seen == 1),
                        stop=(mm_seen == grp["n_mms"]),
                    )
            nc.vector.tensor_copy(
                out=out_sbuf[:out_rows, g * GROUP_W * W:(g + 1) * GROUP_W * W],
                in_=ps[:out_rows, :],
            )
            if per_group is not None:
                per_group(g)


def build_launch_a(plan: Plan, n_table_rows):
    """Launch A: aggregation-1 + GEMMs -> pt [128, NP] (rows 32t+o used)."""
    np_pad = plan.ngroups * GROUP_W * W
    nc = bacc.Bacc("TRN2", target_bir_lowering=False, debug=False, num_devices=8)
    ftab = nc.dram_tensor("ftab", [n_table_rows, 128], BF16, kind="ExternalInput")
    idx_d = nc.dram_tensor("idx", [128, plan.nt * 8], I16, kind="ExternalInput")
    sw_d = nc.dram_tensor("sw", [128, plan.nblk * W], BF16, kind="ExternalInput")
    l1_d = nc.dram_tensor("l1", [4, 128, 128], F32, kind="ExternalInput")  # premasked
    l2_d = nc.dram_tensor("l2", [4, 128, 32], F32, kind="ExternalInput")   # premasked+padded
    pt_d = nc.dram_tensor("pt", [128, np_pad], F32, kind="ExternalOutput")

    hb = SPLIT if n_table_rows > SPLIT else 0
    with tile.TileContext(nc) as tc:
        with (
            tc.tile_pool(name="h0", bufs=1) as h0pool,
            tc.tile_pool(name="wts", bufs=1) as wpool,
            tc.tile_pool(name="hs", bufs=3) as hspool,
            tc.tile_pool(name="ptst", bufs=1) as ptpool,
            tc.tile_pool(name="ps1", bufs=2, space="PSUM") as ps1pool,
            tc.tile_pool(name="ps2", bufs=2, space="PSUM") as ps2pool,
        ):
            h0T = h0pool.tile([128, np_pad], F32)
            w1 = wpool.tile([128, 4, 128], F32)
            nc.sync.dma_start(out=w1[:], in_=l1_d.rearrange("t k h -> k t h"))
            w2 = wpool.tile([128, 4, 32], F32)
            nc.sync.dma_start(out=w2[:], in_=l2_d.rearrange("t k h -> k t h"))
            ptst = ptpool.tile([128, np_pad], F32)

            def per_group(ch):
                sl = slice(ch * 512, (ch + 1) * 512)
                ps2 = ps2pool.tile([128, 512], F32)
                for t in range(4):
                    ps1 = ps1pool.tile([128, 512], F32)
                    nc.tensor.matmul(out=ps1[:], lhsT=w1[:, t, :], rhs=h0T[:, sl],
                                     start=True, stop=True)
                    hs = hspool.tile([128, 512], F32)
                    nc.scalar.activation(
                        out=hs[:], in_=ps1[:],
                        func=mybir.ActivationFunctionType.Relu,
                    )
                    nc.tensor.matmul(out=ps2[32 * t:32 * t + 32, :],
                                     lhsT=w2[:, t, :], rhs=hs[:],
                                     start=True, stop=True,
                                     tile_position=(0, 32 * t))
                nc.vector.tensor_copy(out=ptst[:, sl], in_=ps2[:])
                nc.sync.dma_start(out=pt_d[:, sl], in_=ptst[:, sl])

            emit_aggregation(tc, nc, plan, ftab[:min(SPLIT, n_table_rows), :],
                             ftab[hb:, :], idx_d, sw_d, h0T, 128,
                             per_group=per_group)
    nc.compile()
    return nc


def build_launch_b(plan: Plan, n_table_rows):
    """Launch B: aggregation-2 against p-table -> o2 [64, NP]."""
    np_pad = plan.ngroups * GROUP_W * W
    nc = bacc.Bacc("TRN2", target_bir_lowering=False, debug=False, num_devices=8)
    ptab = nc.dram_tensor("ptab", [n_table_rows, 128], BF16, kind="ExternalInput")
    idx_d = nc.dram_tensor("idx", [128, plan.nt * 8], I16, kind="ExternalInput")
    sw_d = nc.dram_tensor("sw", [128, plan.nblk * W], BF16, kind="ExternalInput")
    o2_d = nc.dram_tensor("o2", [64, np_pad], F32, kind="ExternalOutput")

    hb = SPLIT if n_table_rows > SPLIT else 0
    with tile.TileContext(nc) as tc:
        with tc.tile_pool(name="o2", bufs=1) as opool:
            o2 = opool.tile([64, np_pad], F32)

            def per_group(g):
                sl = slice(g * GROUP_W * W, (g + 1) * GROUP_W * W)
                nc.sync.dma_start(out=o2_d[:, sl], in_=o2[:, sl])

            emit_aggregation(tc, nc, plan, ptab[:min(SPLIT, n_table_rows), :],
                             ptab[hb:, :], idx_d, sw_d, o2, 64,
                             per_group=per_group)
    nc.compile()
    return nc


# ---------------------------------------------------------------------------
# Runners
# ---------------------------------------------------------------------------

def sim_runner(nc, in_maps):
    from concourse.bass_interp import CoreSim
    outs = []
    for m in in_maps:
        sim = CoreSim(nc, trace=False, require_finite=False, require_nnan=False)
        for name, val in m.items():
            sim.tensor(name)[:] = val
        sim.simulate(check_with_hw=False)
        out = {}
        for alloc in nc.m.functions[0].allocations:
            if isinstance(alloc, mybir.MemoryLocationSet) and alloc.kind == "ExternalOutput":
                name = alloc.memorylocations[0].name
                out[name] = np.array(sim.tensor(name))
        outs.append(out)
    return outs


def _install_ntff_hook():
    """The agent image's antenv lacks axon_hooks; synthesize it so
    run_bass_kernel_spmd(trace=True) can NTFF-profile via the axon .so."""
    import types
    if "antenv.axon_hooks" in sys.modules:
        return True
    try:
        from trn_agent_boot.trn_boot import _ntff_profile_via_ctypes
        hook = _ntff_profile_via_ctypes("/opt/axon/libaxon_pjrt.so")
    except Exception:
        return False
    mod = types.ModuleType("antenv.axon_hooks")
    mod._hook = hook
    mod.set_axon_ntff_profile_hook = lambda h: setattr(mod, "_hook", h)
    mod.get_axon_ntff_profile_hook = lambda: mod._hook
    sys.modules["antenv.axon_hooks"] = mod
    try:
        import antenv
        antenv.axon_hooks = mod
    except Exception:
        pass
    return True


def hw_runner_factory(trace=False, label=""):
    from concourse.bass_utils import run_bass_kernel_spmd
    if trace:
        trace = _install_ntff_hook()
    times = {}

    def hw_runner(nc, in_maps):
        res = run_bass_kernel_spmd(nc, in_maps, core_ids=list(range(len(in_maps))),
                                   trace=trace)
        times[label or "t"] = times.get(label or "t", 0) + (res.exec_time_ns or 0)
        hw_runner.last = res
        return res.results

    hw_runner.times = times
    return hw_runner


# ---------------------------------------------------------------------------
# Full host orchestration
# ---------------------------------------------------------------------------

def run(feature, edge_weight, layer1, layer2, src, dst, mask1, mask2,
        n_cores=8, runner=None, trace=False):
    """runner(nc, in_maps) -> list of out dicts; defaults to HW spmd."""
    N = feature.shape[0]
    E = src.shape[0]
    T = mask1.shape[0]
    npc = -(-N // n_cores)          # nodes per core
    nrows = ((N + 127) // 128) * 128
    src = np.asarray(src).astype(np.int64)
    dst = np.asarray(dst).astype(np.int64)
    w = np.asarray(edge_weight).astype(np.float32)

    core_of = dst // npc
    per_core = []
    for k in range(n_cores):
        m = core_of == k
        per_core.append((src[m], dst[m] - k * npc, w[m]))

    real_max = merge_counts([count_core(s, d, npc) for (s, d, _) in per_core])
    plan = Plan(npc, real_max)

    idx_all, sw_all = [], []
    for k in range(n_cores):
        s, d, ww = per_core[k]
        idx_np, sw_np = build_core_data(plan, s, d, ww)
        idx_all.append(idx_np)
        sw_all.append(sw_np)

    # feature table bf16 [nrows, 128]
    ftab = np.zeros((nrows, 128), ml_dtypes.bfloat16)
    ftab[:N] = feature.astype(ml_dtypes.bfloat16)

    # premasked weights
    l1m = (np.asarray(layer1)[None] * np.asarray(mask1)).astype(np.float32)
    l2m = np.zeros((T, 128, 32), np.float32)
    l2m[:, :, :16] = np.asarray(layer2)[None] * np.asarray(mask2)

    nc_a = build_launch_a(plan, nrows)
    in_maps_a = [
        {"ftab": ftab, "idx": idx_all[k], "sw": sw_all[k], "l1": l1m, "l2": l2m}
        for k in range(n_cores)
    ]
    res_a = runner(nc_a, in_maps_a)

    # assemble p-table: rows n -> 64 p values (r = 32t + o from pt rows)
    np_pad = plan.ngroups * GROUP_W * W
    ptab = np.zeros((nrows, 128), ml_dtypes.bfloat16)
    for k in range(n_cores):
        pt = res_a[k]["pt"]  # [128, np_pad]
        rows = np.concatenate([pt[32 * t:32 * t + 16] for t in range(T)])  # [64, NP]
        n0, n1 = k * npc, min((k + 1) * npc, N)
        ptab[n0:n1, :64] = rows[:, : n1 - n0].T.astype(ml_dtypes.bfloat16)

    nc_b = build_launch_b(plan, nrows)
    in_maps_b = [
        {"ptab": ptab, "idx": idx_all[k], "sw": sw_all[k]}
        for k in range(n_cores)
    ]
    res_b = runner(nc_b, in_maps_b)

    out = np.zeros((T, N, 16), np.float32)
    for k in range(n_cores):
        o2 = res_b[k]["o2"]  # [64, np_pad]
        n0, n1 = k * npc, min((k + 1) * npc, N)
        blk = o2[:, : n1 - n0].reshape(T, 16, n1 - n0)
        out[:, n0:n1, :] = blk.transpose(0, 2, 1)
    return out


# ---------------------------------------------------------------------------
# Harness entry point
# ---------------------------------------------------------------------------

def kernel(feature, edge_weight, layer1, layer2, src, dst, mask1, mask2):
    """Full (unsharded) inputs -> full [T, N, 16] float32 output.

    Shards edges by dst range across 8 NeuronCores, runs two Bass launches
    (aggregation-1 + GEMMs, then aggregation-2), gathers on host.
    """
    import os
    trace = bool(os.environ.get("KERNEL_TRACE"))
    runner = hw_runner_factory(trace=trace)
    out = run(
        np.asarray(feature, np.float32),
        np.asarray(edge_weight, np.float32),
        np.asarray(layer1, np.float32),
        np.asarray(layer2, np.float32),
        np.asarray(src),
        np.asarray(dst),
        np.asarray(mask1),
        np.asarray(mask2),
        n_cores=8,
        runner=runner,
    )
    kernel.exec_time_ns = sum(runner.times.values()) if trace else None
    return out
